# revision 5
# baseline (speedup 1.0000x reference)
"""Trainium2 Bass kernel for nn_KinematicLayer: batched forward kinematics.

Full inputs x:[524288,26] f32 -> out:[524288,51] f32.
Data-parallel across 8 NeuronCores (65536 samples/core), 2 chunks/core of
[128 partitions x 256 samples].  Per-sample state tracked as (R 3x3, t 3)
instead of 4x4 homogeneous matmuls; the five limb chains (neck+head, 2 legs,
2 arms) share one instruction stream batched along the free dim (FD=1280).
Trig via half-angle identities keeps every ACT Sin argument inside the
spline's valid [-pi,pi] range:  u=sin(x/2), w=sin(x/4), v=1-2w^2=cos(x/2),
cos=1-2u^2, sin=2uv.  Intermediates are fp16 (DVE 2x mode), outputs fp32.
"""
import numpy as np
import concourse.bass as bass
import concourse.tile as tile
from concourse import bacc, mybir
from concourse.bass_utils import run_bass_kernel_spmd

AF = mybir.ActivationFunctionType
ALU = mybir.AluOpType
f32, f16 = mybir.dt.float32, mybir.dt.float16

N, K, J = 524288, 26, 51
NCORE = 8
NPC = N // NCORE            # 65536 samples per core
FD = 256                    # samples per partition per chunk
CHUNK = 128 * FD            # 32768 samples per chunk
NCHUNK = NPC // CHUNK       # 2

_S = np.array([300.0, 350.0, 75.0, 400.0, 73.96, 249.03, 250.0, 250.0, 170.0],
              np.float32) / 300.0
S0, S1, S2, S3, S4, S5, S6, S7, S8 = [float(v) for v in _S]

# chain order: (neck, Lleg, Rleg, Larm, Rarm); euler angle bases 5,9,13,17,21
# knee-level joints (2,5,8,11,14), distal joints (3,6,9,12,15): both step 3.
DT1 = [S4, -S1, -S1, -S7, -S7]   # signed first-translation lengths
DT2 = [S5, -S0, -S0, -S6, -S6]   # signed distal-translation lengths

FDC = 5 * FD                 # batched chain free dim


def ap_of(t):
    return t[:]


def mk(ap, off, dims):
    """Custom free-dim AP on the same tile/tensor (keeps partition dim)."""
    return bass.AP(ap.tensor, ap.offset + off, [list(ap.ap[0])] + dims)


def build():
    nc = bacc.Bacc("TRN2", target_bir_lowering=False, debug=False,
                   num_devices=NCORE)
    x = nc.dram_tensor("x", [NPC, K], f32, kind="ExternalInput").ap()
    y = nc.dram_tensor("y", [NPC, J], f32, kind="ExternalOutput").ap()

    with tile.TileContext(nc) as tc:
        with (
            tc.tile_pool(name="io", bufs=1) as io,       # X, Y double buffered
            tc.tile_pool(name="per", bufs=1) as per,     # persistent per chunk
            tc.tile_pool(name="scr", bufs=1) as scr,     # small scratch
        ):
            for ch in range(NCHUNK):
                build_chunk(nc, tc, io, per, scr, x, y, ch)
    nc.compile()
    return nc


def build_chunk(nc, tc, io, per, scr, x, y, ch):
    V, A = nc.vector, nc.scalar
    base = ch * CHUNK

    X = io.tile([128, K * FD], f32, tag="X")
    nc.gpsimd.dma_start(X[:], bass.AP(x.tensor, base * K,
                                      [[FD * K, 128], [1, K * FD]]))
    Y = io.tile([128, J * FD], f32, tag="Y")
    Xa = X[:]
    Ya = Y[:]

    def xang(k):                       # angle k strided view [128, FD]
        return mk(Xa, k, [[K, FD]])

    def ycol(c):                       # output scalar col c (0..50) strided
        return mk(Ya, c, [[J, FD]])

    def ygrp(c0):                      # batched 5-chain joint write, offset c0
        return mk(Ya, c0, [[9, 5], [J, FD]])

    # ---------------- trig: 6 groups ----------------
    # group APs reading X: pelvis/torso = angles 0..4 step 1; chain pos j =
    # angles 5+j step 4 across chains.
    def trig(tag, xap, n):
        fd = n * FD
        u = scr.tile([128, fd], f16, tag="trigU", name="trigU")
        w = scr.tile([128, fd], f16, tag="trigW", name="trigW")
        A.activation(u[:], xap, AF.Sin, scale=0.5)
        A.activation(w[:], xap, AF.Sin, scale=0.25)
        q = scr.tile([128, fd], f16, tag="trigQ", name="trigQ")
        c = per.tile([128, fd], f16, tag=f"C{tag}", name=f"C{tag}")
        s = per.tile([128, fd], f16, tag=f"S{tag}", name=f"S{tag}")
        V.tensor_tensor(q[:], u[:], u[:], ALU.mult)
        V.tensor_scalar(c[:], q[:], -2.0, 1.0, ALU.mult, ALU.add)
        V.tensor_tensor(q[:], w[:], w[:], ALU.mult)
        V.tensor_scalar(q[:], q[:], -2.0, 1.0, ALU.mult, ALU.add)  # v in q
        V.scalar_tensor_tensor(s[:], u[:], 2.0, q[:], ALU.mult, ALU.mult)
        return c, s

    Cpt, Spt = trig("pt", mk(Xa, 0, [[1, 5], [K, FD]]), 5)
    CS = [trig(f"p{j}", mk(Xa, 5 + j, [[4, 5], [K, FD]]), 5) for j in range(4)]

    def pt(t, i):                      # pelvis/torso angle slice i of 0..4
        return t[:, i * FD:(i + 1) * FD]

    c0, s0 = pt(Cpt, 0), pt(Spt, 0)
    c1, s1 = pt(Cpt, 1), pt(Spt, 1)
    c2, s2 = pt(Cpt, 2), pt(Spt, 2)
    c3, s3 = pt(Cpt, 3), pt(Spt, 3)
    c4, s4 = pt(Cpt, 4), pt(Spt, 4)

    def tt(out, a, b, op):
        V.tensor_tensor(out, a, b, op)

    def fresh(tag, fd=FD, dt=f16, pool=None):
        return (pool or scr).tile([128, fd], dt, tag=tag, name=tag)

    def mul(a, b, tag="m", fd=FD):
        o = fresh(tag, fd=fd)
        tt(o[:], a, b, ALU.mult)
        return o[:]

    def nmul(a, b, tag="m"):           # -(a*b)
        o = fresh(tag)
        V.scalar_tensor_tensor(o[:], a, -1.0, b, ALU.mult, ALU.mult)
        return o[:]

    def comb(a, b, op, tag="m", pool=None, fd=FD):
        o = fresh(tag, fd=fd, pool=pool)
        tt(o[:], a, b, op)
        return o[:]

    # ---------------- pelvis R ----------------
    ms0s1 = mul(s0, s1, "ms01")
    mc0s1 = mul(c0, s1, "mc01")
    P1x = nmul(s0, c1, "P1x")
    P1y = mul(c0, c1, "P1y")
    P1z = s1                                        # alias
    P0x = comb(mul(c0, c2), mul(ms0s1, s2, "m2"), ALU.subtract, "P0x", per)
    P0y = comb(mul(s0, c2), mul(mc0s1, s2, "m2"), ALU.add, "P0y", per)
    P0z = nmul(c1, s2, "P0z")
    P2x = comb(mul(c0, s2), mul(ms0s1, c2, "m2"), ALU.add, "P2x", per)
    P2y = comb(mul(s0, s2), mul(mc0s1, c2, "m2"), ALU.subtract, "P2y", per)
    P2z = mul(c1, c2, "P2z")
    P0 = (P0x, P0y, P0z)
    P1 = (P1x, P1y, P1z)
    P2 = (P2x, P2y, P2z)

    # ---------------- torso R = Rpel @ Rz3 @ Ry4 ----------------
    def colupd(cc, ss, A3, B3, tagp, pool=None, fd=FD):
        """returns cc*A + ss*B per component."""
        out = []
        for i, (a, b) in enumerate(zip(A3, B3)):
            out.append(comb(mul(cc, a, "ca", fd), mul(ss, b, "cb", fd), ALU.add,
                            f"{tagp}{i}", pool, fd))
        return tuple(out)

    def colupd_sub(cc, ss, A3, B3, tagp, pool=None, fd=FD):
        """returns cc*A - ss*B per component."""
        out = []
        for i, (a, b) in enumerate(zip(A3, B3)):
            out.append(comb(mul(cc, a, "ca", fd), mul(ss, b, "cb", fd), ALU.subtract,
                            f"{tagp}{i}", pool, fd))
        return tuple(out)

    D0t = colupd(c3, s3, P0, P1, "D0t")
    D1t = colupd_sub(c3, s3, P1, P0, "D1t", per)       # E1 = D1t
    E0 = colupd_sub(c4, s4, D0t, P2, "E0", per)
    E2 = colupd(s4, c4, D0t, P2, "E2", per)

    # ---------------- phase A translations ----------------
    scH = fresh("scH")
    V.tensor_copy(scH[:], mk(Xa, 25, [[K, FD]]))       # scale as fp16

    TP = [per.tile([128, FDC], f16, tag=f"TP{c}", name=f"TP{c}") for c in range(3)]

    def tp_slice(c, i):
        return TP[c][:, i * FD:(i + 1) * FD]

    for c in range(3):
        # torso t = S3*scale*D1 -> Y joint1 + TP[neck]
        V.scalar_tensor_tensor(ycol(3 * 1 + c), scH[:], S3, D1t[c],
                               ALU.mult, ALU.mult)
        V.scalar_tensor_tensor(tp_slice(c, 0), scH[:], S3, D1t[c],
                               ALU.mult, ALU.mult)
        # hips: +-S2*scale*P0 -> TP legs + Y joints 4,7
        V.scalar_tensor_tensor(tp_slice(c, 1), scH[:], S2, P0[c],
                               ALU.mult, ALU.mult)
        V.scalar_tensor_tensor(tp_slice(c, 2), scH[:], -S2, P0[c],
                               ALU.mult, ALU.mult)
        A.copy(ycol(3 * 4 + c), tp_slice(c, 1))
        A.copy(ycol(3 * 7 + c), tp_slice(c, 2))
        # shoulders: t_tor +- S8*scale*E0 -> TP arms + Y joints 10,13
        u = fresh("shu")
        V.scalar_tensor_tensor(u[:], scH[:], S8, E0[c], ALU.mult, ALU.mult)
        tt(tp_slice(c, 3), tp_slice(c, 0), u[:], ALU.add)
        tt(tp_slice(c, 4), tp_slice(c, 0), u[:], ALU.subtract)
        A.copy(ycol(3 * 10 + c), tp_slice(c, 3))
        A.copy(ycol(3 * 13 + c), tp_slice(c, 4))
    # pelvis joint 0 = 0
    V.memset(mk(Ya, 0, [[J, FD], [1, 3]]), 0.0)

    # ---------------- batched parent-R tiles ----------------
    # chains: 0=neck(E), 1,2=legs(P), 3,4=arms(E)
    PR = [[per.tile([128, FDC], f16, tag=f"PR{c}{i}", name=f"PR{c}{i}") for i in range(3)]
          for c in range(3)]
    for ci, (Ecol, Pcol) in enumerate(((E0, P0), (D1t, P1), (E2, P2))):
        for i in range(3):
            dst = PR[ci][i][:]
            e = Ecol[i]
            p = Pcol[i]
            def bc2(src):
                return bass.AP(src.tensor, src.offset,
                               [list(src.ap[0]), [0, 2], [1, FD]])
            V.tensor_copy(mk(dst, 0, [[1, FD]]), e)
            V.tensor_copy(mk(dst, FD, [[1, 2 * FD]]), bc2(p))
            V.tensor_copy(mk(dst, 3 * FD, [[1, 2 * FD]]), bc2(e))
    PR0, PR1, PR2 = PR

    def prc(c):
        return tuple(PR[c][i][:] for i in range(3))

    cA, sA = (t[:] for t in CS[0])
    cB, sB = (t[:] for t in CS[1])
    cG, sG = (t[:] for t in CS[2])
    cD, sD = (t[:] for t in CS[3])

    # ---------------- batched chain (FD=1280 ops) ----------------
    bD0 = colupd(cA, sA, prc(0), prc(1), "bD0", per, FDC)
    bD1 = colupd_sub(cA, sA, prc(1), prc(0), "bD1", per, FDC)
    bK1 = colupd(cB, sB, bD1, prc(2), "bK1", per, FDC)
    bK2 = colupd_sub(cB, sB, prc(2), bD1, "bK2", per, FDC)
    bK2p = colupd(sG, cG, bD0, bK2, "bD1", per, FDC)  # reuse bD1 slots
    bC1 = colupd(cD, sD, bK1, bK2p, "bD0", per, FDC)  # reuse bD0 slots

    # dT tiles: per-chain signed bone length * scale
    scB = fresh("scB", FDC)
    V.tensor_copy(scB[:], mk(scH[:], 0, [[0, 5], [1, FD]]))
    dT1 = fresh("dT1", FDC)
    dT2 = fresh("dT2", FDC)
    for i in range(5):
        sl = slice(i * FD, (i + 1) * FD)
        V.tensor_scalar(dT1[:, sl], scB[:, sl], DT1[i], None, ALU.mult)
        V.tensor_scalar(dT2[:, sl], scB[:, sl], DT2[i], None, ALU.mult)

    for c in range(3):
        u = fresh("btr", FDC)
        tt(u[:], dT1[:], bK1[c], ALU.mult)
        tt(ygrp(3 * 2 + c), TP[c][:], u[:], ALU.add)       # knee-level joints
        u2 = fresh("btr2", FDC)
        tt(u2[:], dT2[:], bC1[c], ALU.mult)
        tt(ygrp(3 * 3 + c), ygrp(3 * 2 + c), u2[:], ALU.add)  # distal joints

    # ---------------- thorax = 0.5*(p8 + p6) ----------------
    for c in range(3):
        h = fresh("thx")
        tt(h[:], ycol(3 * 8 + c), ycol(3 * 6 + c), ALU.add)
        V.tensor_scalar(ycol(48 + c), h[:], 0.5, None, ALU.mult)

    nc.gpsimd.dma_start(bass.AP(y.tensor, base * J,
                                [[FD * J, 128], [1, J * FD]]), Y[:])


_NC = None


def kernel(x: np.ndarray) -> np.ndarray:
    global _NC
    if _NC is None:
        _NC = build()
    x = np.ascontiguousarray(x, dtype=np.float32)
    shards = x.reshape(NCORE, NPC, K)
    res = run_bass_kernel_spmd(
        _NC, [{"x": shards[i]} for i in range(NCORE)],
        core_ids=list(range(NCORE)))
    return np.concatenate([r["y"] for r in res.results], axis=0)
